# revision 40
# baseline (speedup 1.0000x reference)
"""Causal self-attention (B=2, T=2048, C=1024, H=16, D=64) on 8 trn2 cores.

Sharding: core = b*4 + hg  (data-parallel over batch b, tensor-parallel over
4 head-groups of 4 heads).  Each core computes q/k/v projections for its
256 head-dims, flash-style causal attention for its 4 heads, and a partial
output projection (its 256-column slice of Wp).  Partials are summed on the
host (the all-reduce), bias bp added there too.

Device layout notes (v2):
  - All tensors bf16 in DRAM/SBUF (half DMA + SBUF), fp32 PSUM accumulate.
    End-to-end bf16 pipeline error measured 4.1e-3 max-rel on host.
  - kT stored zero-padded per head: kz[h] = [128, T] with head h's 64 dims
    at partition offset (h%2)*64 and ZEROS elsewhere, so S matmuls run with
    K=128 (full-partition stream ~0.45 ns/row vs 0.62 at K=64) against the
    2-heads-packed qT tile: the zero weight rows kill the other head.
  - Causal mask applied inside PSUM: the diagonal 128-wide block of each
    S chunk is computed as its own accumulation group and an
    identity-weights matmul adds trineg (-240 above the diagonal) before
    stop; exp then yields ~e-26 ~ 0 there.  No DVE op on the S->PV path.
  - v stored [t, d'] with a ones-column per head (65 cols/head) so the PV
    matmul's row 64 is the softmax denominator l[q] for free.
  - S^T tiles [k=128, q<=512] in PSUM, exp on ACT (scale=1/8 fused, no
    max-subtraction: |S/8|<~5 so exp is safe), pt in bf16.
  - The attention loop is software-pipelined: S matmuls for group g+1 are
    emitted before PV of group g; projections for tile j+1 and deferred
    out-projections fill remaining PE slots (keeps the HAM clock warm).
  - Normalization: l -> PE partition-broadcast -> reciprocal_approx_fast
    -> DVE mul into yT.
"""
import numpy as np
import ml_dtypes
from contextlib import ExitStack

B, T, C, H, D = 2, 2048, 1024, 16, 64
HLOC = 4            # heads per core
CLOC = HLOC * D     # 256 head-dims per core
VW = HLOC * 65      # v width with ones-columns: 260
N_CORES = 8
TQ = 512            # q tile width
KC = 128            # k chunk
NCC = C // 128      # 8 contraction chunks

_CACHE = {}


def build_nc(with_qk_bias=True):
    import concourse.tile as tile
    from concourse import bacc, mybir

    f32 = mybir.dt.float32
    fmm = mybir.dt.bfloat16
    Exp = mybir.ActivationFunctionType.Exp

    nc = bacc.Bacc("TRN2", target_bir_lowering=False, debug=False,
                   num_devices=N_CORES)
    xT = nc.dram_tensor("xT", [C, T], fmm, kind="ExternalInput").ap()
    wqkT = nc.dram_tensor("wqkT", [C, 2 * CLOC], fmm, kind="ExternalInput").ap()
    wvT = nc.dram_tensor("wvT", [C, VW], fmm, kind="ExternalInput").ap()
    wpT = nc.dram_tensor("wpT", [CLOC, C], fmm, kind="ExternalInput").ap()
    if with_qk_bias:
        bq = nc.dram_tensor("bq", [1, CLOC], fmm, kind="ExternalInput").ap()
        bk = nc.dram_tensor("bk", [1, CLOC], fmm, kind="ExternalInput").ap()
    bv = nc.dram_tensor("bv", [1, VW], fmm, kind="ExternalInput").ap()
    trineg = nc.dram_tensor("trineg", [KC, KC], fmm, kind="ExternalInput").ap()
    ident = nc.dram_tensor("ident", [KC, KC], fmm, kind="ExternalInput").ap()
    po = nc.dram_tensor("po", [T, C], fmm, kind="ExternalOutput").ap()

    with tile.TileContext(nc) as tc, ExitStack() as ctx:
        persist = ctx.enter_context(tc.tile_pool(name="persist", bufs=1))
        pt_pool = ctx.enter_context(tc.tile_pool(name="pt", bufs=5))
        stage = ctx.enter_context(tc.tile_pool(name="stage", bufs=4))
        norm_pool = ctx.enter_context(tc.tile_pool(name="norm", bufs=4))
        ps_small = ctx.enter_context(
            tc.tile_pool(name="ps_small", bufs=2, space="PSUM"))
        ps_st = ctx.enter_context(
            tc.tile_pool(name="ps_st", bufs=2, space="PSUM"))
        ps_yt = ctx.enter_context(
            tc.tile_pool(name="ps_yt", bufs=2, space="PSUM"))

        # ---- persistent SBUF tensors + loads ----
        xT_sb = [persist.tile([128, T], fmm, tag=f"xT{c}", name=f"xT{c}") for c in range(NCC)]
        wqk_sb = [persist.tile([128, 2 * CLOC], fmm, tag=f"wqk{c}", name=f"wqk{c}") for c in range(NCC)]
        wv_sb = [persist.tile([128, VW], fmm, tag=f"wv{c}", name=f"wv{c}") for c in range(NCC)]
        wp_sb = [persist.tile([128, C], fmm, tag=f"wp{m}", name=f"wp{m}") for m in range(2)]
        if with_qk_bias:
            bq_sb = persist.tile([1, CLOC], fmm, tag="bq")
            bk_sb = persist.tile([1, CLOC], fmm, tag="bk")
        # bv / onesrow are [128, *] with only row 0 nonzero: broadcasts run
        # as K=128 matmuls (full-rate stream) instead of K=1 (~0.6 ns/row)
        bv_sb = persist.tile([128, VW], fmm, tag="bv")
        onesrow_sb = persist.tile([128, 128], fmm, tag="onesrow")
        ones_sb = persist.tile([1, TQ], fmm, tag="ones")
        trineg_sb = persist.tile([KC, KC], fmm, tag="trineg")
        ident_sb = persist.tile([KC, KC], fmm, tag="ident")
        qT_sb = [persist.tile([128, T], fmm, tag=f"qT{m}", name=f"qT{m}") for m in range(2)]
        kz_sb = [persist.tile([128, T], fmm, tag=f"kz{h}", name=f"kz{h}") for h in range(HLOC)]
        v_sb = [persist.tile([128, VW], fmm, tag=f"v{t}", name=f"v{t}") for t in range(T // 128)]
        yT_sb = [persist.tile([128, T], fmm, tag=f"yT{m}", name=f"yT{m}") for m in range(2)]

        # zero halves of kz (the other head's partitions stay 0 forever);
        # DVE is idle during the initial DMA wait.  ones via memset, not DMA.
        for h in range(HLOC):
            zsl = slice(64, 128) if h % 2 == 0 else slice(0, 64)
            nc.vector.memset(kz_sb[h][zsl, :], 0.0)
        nc.gpsimd.memset(ones_sb[:], 1.0)
        nc.gpsimd.memset(onesrow_sb[:], 0.0)
        nc.gpsimd.memset(onesrow_sb[0:1, :], 1.0)
        nc.gpsimd.memset(bv_sb[:], 0.0)
        # Input loads split across the three DMA-capable queues (SP / ACT /
        # GPSIMD) so the ~5 MB prologue pull isn't serialized on one ring.
        # sync carries the fused qk weights, scalar the x tile the first
        # projections read, gpsimd the mask constants + v weights; the
        # later x tiles trail on whichever queue frees up first.
        # wqkT host layout: [wq_m0 | wk_m0 | wq_m1 | wk_m1] per chunk, so
        # only the first 256 cols (the m0 half) gate the first projections.
        for c in range(NCC):
            sl = slice(c * 128, (c + 1) * 128)
            nc.sync.dma_start(wqk_sb[c][:, 0:2 * 128], wqkT[sl, 0:2 * 128])
            if c < 4:
                nc.scalar.dma_start(xT_sb[c][:, 0:TQ], xT[sl, 0:TQ])
            else:
                nc.gpsimd.dma_start(xT_sb[c][:, 0:TQ], xT[sl, 0:TQ])
        for c in range(NCC):
            sl = slice(c * 128, (c + 1) * 128)
            eng = nc.scalar if c < 4 else nc.gpsimd
            eng.dma_start(wv_sb[c][:], wvT[sl, :])
        nc.gpsimd.dma_start(bv_sb[0:1, :], bv[:])
        nc.gpsimd.dma_start(trineg_sb[:], trineg[:])
        nc.gpsimd.dma_start(ident_sb[:], ident[:])
        for c in range(NCC):
            sl = slice(c * 128, (c + 1) * 128)
            nc.sync.dma_start(wqk_sb[c][:, 2 * 128:4 * 128],
                              wqkT[sl, 2 * 128:4 * 128])
        # Background tiles stay OFF the scalar queue: ACT issues DMAs
        # in-order ahead of the exps, so anything queued there delays the
        # first softmax.  xT1/xT2 on sync, xT3+wp on gpsimd.
        for c in range(NCC):
            sl = slice(c * 128, (c + 1) * 128)
            nc.sync.dma_start(xT_sb[c][:, TQ:2 * TQ], xT[sl, TQ:2 * TQ])
        for c in range(NCC):
            sl = slice(c * 128, (c + 1) * 128)
            nc.sync.dma_start(xT_sb[c][:, 2 * TQ:3 * TQ], xT[sl, 2 * TQ:3 * TQ])
        for c in range(NCC):
            sl = slice(c * 128, (c + 1) * 128)
            nc.gpsimd.dma_start(xT_sb[c][:, 3 * TQ:4 * TQ], xT[sl, 3 * TQ:4 * TQ])
        for m in range(2):
            nc.gpsimd.dma_start(wp_sb[m][:], wpT[m * 128:(m + 1) * 128, :])
        if with_qk_bias:
            nc.sync.dma_start(bq_sb[:], bq[:])
            nc.sync.dma_start(bk_sb[:], bk[:])

        # ---- interleaved emission: projections / attention / out-proj ----
        # The PE executes its queue in order, so emission order controls PE
        # density.  Attention for q-tile j only needs projections up to
        # t=j, so projections for t=j+1 and the out-projection are woven
        # between attention groups of j to fill PE idle slots.
        def proj_qk(is_k, m, t):
            tsl = slice(t * TQ, (t + 1) * TQ)
            blk = 2 * m + (1 if is_k else 0)   # [q_m0|k_m0|q_m1|k_m1]
            wsl = slice(blk * 128, (blk + 1) * 128)
            msl = slice(m * 128, (m + 1) * 128)
            ps = ps_small.tile([128, TQ], f32, tag="ps_small")
            for c in range(NCC):
                nc.tensor.matmul(ps[:], wqk_sb[c][:, wsl], xT_sb[c][:, tsl],
                                 start=(c == 0),
                                 stop=(c == NCC - 1 and not with_qk_bias))
            if with_qk_bias:
                b_sb = bk_sb if is_k else bq_sb
                nc.tensor.matmul(ps[:], b_sb[0:1, msl], ones_sb[0:1, :],
                                 start=False, stop=True)
            if is_k:
                nc.vector.tensor_copy(kz_sb[2 * m][0:64, tsl], ps[0:64, :])
                nc.vector.tensor_copy(kz_sb[2 * m + 1][64:128, tsl],
                                      ps[64:128, :])
            else:
                nc.vector.tensor_copy(qT_sb[m][:, tsl], ps[:])

        def proj_v(tt):
            ttsl = slice(tt * 128, tt * 128 + 128)
            ps = ps_small.tile([128, VW], f32, tag="ps_small")
            for c in range(NCC):
                nc.tensor.matmul(ps[:], xT_sb[c][:, ttsl], wv_sb[c][:],
                                 start=(c == 0), stop=False)
            # always emitted: supplies the ones-columns (+ v bias)
            nc.tensor.matmul(ps[:], onesrow_sb[:], bv_sb[:],
                             start=False, stop=True)
            nc.vector.tensor_copy(v_sb[tt][:], ps[:])

        def proj_pieces(t):
            out = []
            for is_k in (False, True):
                for m in range(2):
                    out.append(lambda k=is_k, mm=m: proj_qk(k, mm, t))
            for tt in range(t * 4, t * 4 + 4):
                out.append(lambda x=tt: proj_v(x))
            return out

        def outproj_piece(tt, do, alt=False, act_copy=False):
            ttsl = slice(tt * 128, (tt + 1) * 128)
            dsl = slice(do * TQ, (do + 1) * TQ)
            if alt:   # epilogue: borrow the idle attention PSUM/ACT
                ops = ps_st.tile([128, TQ], f32, tag="st")
            else:
                ops = ps_small.tile([128, TQ], f32, tag="ps_small")
            for m2 in range(2):
                nc.tensor.matmul(ops[:], yT_sb[m2][:, ttsl],
                                 wp_sb[m2][:, dsl],
                                 start=(m2 == 0), stop=(m2 == 1))
            so = stage.tile([128, TQ], fmm, tag="so2" if alt else "so")
            if alt:
                nc.scalar.copy(so[:], ops[:])
                nc.gpsimd.dma_start(po[ttsl, dsl], so[:])
            elif act_copy:
                nc.scalar.copy(so[:], ops[:])
                nc.sync.dma_start(po[ttsl, dsl], so[:])
            else:
                nc.vector.tensor_copy(so[:], ops[:])
                nc.sync.dma_start(po[ttsl, dsl], so[:])

        def outproj_pieces(j, act_every=0):
            ts = [(tt, do) for tt in range(4 * j, 4 * j + 4) for do in range(2)]
            return [lambda t=tt, d=do, a=(act_every and pi % act_every == act_every - 1):
                    outproj_piece(t, d, act_copy=a)
                    for pi, (tt, do) in enumerate(ts)]

        def s_group(j, h, kcs):
            """Emit S matmuls for a k-chunk pair; return (st_tile, info).

            K=128 against zero-padded kz.  Diagonal chunks are split so the
            128-wide diagonal block forms its own accumulation group with a
            trineg (-240 upper triangle) matmul-add before stop.
            """
            m = h // 2
            st = ps_st.tile([128, 1024], f32, tag="st")
            info = []
            for i, kc in enumerate(kcs):
                coff = max(0, kc * KC - j * TQ)   # causal column offset
                lo = i * TQ + coff
                qlo = j * TQ + coff
                if kc >= 4 * j:   # diagonal chunk
                    nc.tensor.matmul(
                        st[:, lo:lo + KC],
                        kz_sb[h][:, kc * KC:(kc + 1) * KC],
                        qT_sb[m][:, qlo:qlo + KC],
                        start=True, stop=False)
                    nc.tensor.matmul(
                        st[:, lo:lo + KC], ident_sb[:], trineg_sb[:],
                        start=False, stop=True)
                    if (i + 1) * TQ > lo + KC:
                        nc.tensor.matmul(
                            st[:, lo + KC:(i + 1) * TQ],
                            kz_sb[h][:, kc * KC:(kc + 1) * KC],
                            qT_sb[m][:, qlo + KC:(j + 1) * TQ],
                            start=True, stop=True)
                else:
                    nc.tensor.matmul(
                        st[:, lo:(i + 1) * TQ],
                        kz_sb[h][:, kc * KC:(kc + 1) * KC],
                        qT_sb[m][:, qlo:(j + 1) * TQ],
                        start=True, stop=True)
                info.append((i, kc, coff))
            return st, info

        def pv_group(j, h, st, info, yt, nk, split_exp=False):
            """exp + PV matmuls for a prepared S group (mask already in st).

            split_exp: emit one exp per chunk instead of merged runs so the
            first PV matmul's dependency resolves ~0.5us earlier (used for
            the first group after a head transition, where the software
            pipeline has the least slack).
            """
            pt = pt_pool.tile([128, 1024], fmm, tag="pt")
            runs = []
            for i, kc, coff in info:
                lo, hi = i * TQ + coff, (i + 1) * TQ
                if not split_exp and runs and runs[-1][1] == lo:
                    runs[-1][1] = hi
                else:
                    runs.append([lo, hi])
            for lo, hi in runs:
                nc.scalar.activation(pt[:, lo:hi], st[:, lo:hi], Exp, scale=0.125)
            for i, kc, coff in info:
                lo = i * TQ + coff
                nc.tensor.matmul(
                    yt[0:65, coff:TQ] if coff else yt[:],
                    v_sb[kc][:, h * 65:(h + 1) * 65],
                    pt[:, lo:(i + 1) * TQ],
                    start=(kc == 0), stop=(kc == nk - 1))

        # l tiles are [128, TQ] with rows 1..127 pre-zeroed (once, below) so
        # the broadcast matmul runs K=128 full-rate; only row 0 is rewritten
        for _ in range(4):
            lz = norm_pool.tile([128, TQ], fmm, tag="l", name="lz")
            nc.vector.memset(lz[:], 0.0)

        def norm_a(yt, eng=None):
            """Pull the softmax denominator out of PSUM right away."""
            l_sb = norm_pool.tile([128, TQ], fmm, tag="l")
            if eng == "scalar":
                nc.scalar.copy(l_sb[0:1, :], yt[64:65, :])
            else:
                nc.vector.tensor_copy(l_sb[0:1, :], yt[64:65, :])
            return l_sb

        def norm_b(j, h, yt, l_sb):
            """yT[h slice, j] = yt[0:64] * broadcast(1/l)."""
            m, pr = h // 2, (h % 2) * 64
            bc_ps = ps_small.tile([64, TQ], f32, tag="ps_small")
            nc.tensor.matmul(bc_ps[:], onesrow_sb[:, 0:64], l_sb[:],
                             start=True, stop=True)
            bc_sb = stage.tile([64, TQ], f32, tag="bc")
            nc.vector.reciprocal_approx_fast(bc_sb[:], bc_ps[:])
            nc.vector.tensor_mul(yT_sb[m][pr:pr + 64, j * TQ:(j + 1) * TQ],
                                 yt[0:64, :], bc_sb[:])

        # prologue: only what the first attention groups need (m0 heads,
        # first two v tiles); the rest weaves into j=0 as extras so S(h0)
        # starts as soon as the critical DMA lands
        proj_qk(False, 0, 0)
        proj_qk(True, 0, 0)
        proj_v(0)
        proj_v(1)

        for j in range(T // TQ):
            nk = 4 * (j + 1)
            groups = []
            for h in range(HLOC):
                for k0 in range(0, nk, 2):
                    groups.append((h, [k for k in (k0, k0 + 1) if k < nk],
                                   k0 == 0))
            extras = []
            if j == 0:
                extras += [lambda: proj_v(2), lambda: proj_v(3),
                           lambda: proj_qk(False, 1, 0),
                           lambda: proj_qk(True, 1, 0)]
            if j + 1 < T // TQ:
                extras += proj_pieces(j + 1)
            if j == T // TQ - 1:
                # all deferred out-projections: PE filler for the
                # ACT-paced final block (keeps HAM warm)
                for jj in range(T // TQ - 1):
                    extras += outproj_pieces(jj)
            ei = 0           # extras emitted so far
            yts = {}
            pending = None   # (h, st, info, first) awaiting exp/PV
            norm_w = []      # [age, h, yt, l_sb] awaiting broadcast/mul
            for gi, (h, kcs, first) in enumerate(groups):
                for e in norm_w:
                    e[0] += 1
                if norm_w and norm_w[0][0] >= 3:
                    _, nh, nyt, nl = norm_w.pop(0)
                    norm_b(j, nh, nyt, nl)
                if h not in yts:
                    yts[h] = ps_yt.tile([65, TQ], f32, tag="yt",
                                        name=f"yt{j}_{h}")
                st, info = s_group(j, h, kcs)
                if pending is not None:
                    ph, pst, pinfo, pfirst = pending
                    pv_group(j, ph, pst, pinfo, yts[ph], nk, pfirst)
                    if ph != h:
                        yt = yts.pop(ph)
                        norm_w.append([0, ph, yt, norm_a(yt)])
                pending = (h, st, info, first)
                want = (gi + 1) * len(extras) // len(groups)
                while ei < want:
                    extras[ei]()
                    ei += 1
            ph, pst, pinfo, pfirst = pending
            pv_group(j, ph, pst, pinfo, yts[ph], nk, pfirst)
            yt = yts.pop(ph)
            for e in norm_w:
                e[0] += 1
            norm_w.append([0, ph, yt, norm_a(yt)])
            # aged heads' l is long ready: normalize them before the
            # leftover extras; the just-finished head after (l-copy hides
            # under the extras' PE work)
            while norm_w and norm_w[0][0] >= 1:
                _, nh, nyt, nl = norm_w.pop(0)
                norm_b(j, nh, nyt, nl)
            while ei < len(extras):
                extras[ei]()
                ei += 1
            for _, nh, nyt, nl in norm_w:
                norm_b(j, nh, nyt, nl)

        for pi, (tt, do) in enumerate(
                [(t, d) for t in range(4 * (T // TQ - 1), 4 * (T // TQ))
                 for d in range(2)]):   # epilogue
            outproj_piece(tt, do, alt=(pi % 2 == 1))
    nc.compile()
    return nc


def make_in_maps(x, Wq, bq, Wk, bk, Wv, bv, Wp, bp):
    bf16 = ml_dtypes.bfloat16
    x = np.asarray(x, np.float32)
    Wq, Wk, Wv, Wp = (np.asarray(w, np.float32) for w in (Wq, Wk, Wv, Wp))
    bq, bk, bv = (np.asarray(b, np.float32) for b in (bq, bk, bv))

    kp = np.arange(KC)[:, None]
    qf = np.arange(KC)[None, :]
    trineg = np.where(kp > qf, -240.0, 0.0).astype(bf16)
    ident = np.eye(KC, dtype=bf16)
    with_qk_bias = bool(np.any(bq) or np.any(bk))

    in_maps = []
    for core in range(N_CORES):
        b = core // 4
        hg = core % 4
        rows = slice(hg * CLOC, (hg + 1) * CLOC)
        wv_aug = np.zeros((C, VW), np.float32)
        bv_aug = np.zeros((1, VW), np.float32)
        for h in range(HLOC):
            wsl = slice(hg * CLOC + h * D, hg * CLOC + (h + 1) * D)
            wv_aug[:, h * 65:h * 65 + D] = Wv[wsl, :].T
            bv_aug[0, h * 65:h * 65 + D] = bv[wsl]
            bv_aug[0, h * 65 + D] = 1.0
        im = {
            "xT": np.ascontiguousarray(x[b].T).astype(bf16),
            "wqkT": np.ascontiguousarray(
                np.concatenate([Wq[rows, :].T[:, 0:128], Wk[rows, :].T[:, 0:128],
                                Wq[rows, :].T[:, 128:256], Wk[rows, :].T[:, 128:256]],
                               axis=1)
            ).astype(bf16),
            "wvT": wv_aug.astype(bf16),
            "wpT": np.ascontiguousarray(Wp[:, rows].T).astype(bf16),
            "bv": bv_aug.astype(bf16),
            "trineg": trineg,
            "ident": ident,
        }
        if with_qk_bias:
            im["bq"] = np.ascontiguousarray(bq[rows][None, :]).astype(bf16)
            im["bk"] = np.ascontiguousarray(bk[rows][None, :]).astype(bf16)
        in_maps.append(im)
    return in_maps


def kernel(x, Wq, bq, Wk, bk, Wv, bv, Wp, bp):
    from concourse.bass_utils import run_bass_kernel_spmd

    with_qk_bias = bool(np.any(np.asarray(bq)) or np.any(np.asarray(bk)))
    key = ("nc", with_qk_bias)
    if key not in _CACHE:
        _CACHE[key] = build_nc(with_qk_bias)
    nc = _CACHE[key]
    in_maps = make_in_maps(x, Wq, bq, Wk, bk, Wv, bv, Wp, bp)
    res = run_bass_kernel_spmd(nc, in_maps, core_ids=list(range(N_CORES)))
    out = np.zeros((B, T, C), np.float32)
    for core in range(N_CORES):
        out[core // 4] += np.asarray(res.results[core]["po"], np.float32)
    out += np.asarray(bp, np.float32)[None, None, :]
    return out


# revision 41
# speedup vs baseline: 1.0107x; 1.0107x over previous
"""Causal self-attention (B=2, T=2048, C=1024, H=16, D=64) on 8 trn2 cores.

Sharding: core = b*4 + hg  (data-parallel over batch b, tensor-parallel over
4 head-groups of 4 heads).  Each core computes q/k/v projections for its
256 head-dims, flash-style causal attention for its 4 heads, and a partial
output projection (its 256-column slice of Wp).  Partials are summed on the
host (the all-reduce), bias bp added there too.

Device layout notes (v2):
  - All tensors bf16 in DRAM/SBUF (half DMA + SBUF), fp32 PSUM accumulate.
    End-to-end bf16 pipeline error measured 4.1e-3 max-rel on host.
  - kT stored zero-padded per head: kz[h] = [128, T] with head h's 64 dims
    at partition offset (h%2)*64 and ZEROS elsewhere, so S matmuls run with
    K=128 (full-partition stream ~0.45 ns/row vs 0.62 at K=64) against the
    2-heads-packed qT tile: the zero weight rows kill the other head.
  - Causal mask applied inside PSUM: the diagonal 128-wide block of each
    S chunk is computed as its own accumulation group and an
    identity-weights matmul adds trineg (-240 above the diagonal) before
    stop; exp then yields ~e-26 ~ 0 there.  No DVE op on the S->PV path.
  - v stored [t, d'] with a ones-column per head (65 cols/head) so the PV
    matmul's row 64 is the softmax denominator l[q] for free.
  - S^T tiles [k=128, q<=512] in PSUM, exp on ACT (scale=1/8 fused, no
    max-subtraction: |S/8|<~5 so exp is safe), pt in bf16.
  - The attention loop is software-pipelined: S matmuls for group g+1 are
    emitted before PV of group g; projections for tile j+1 and deferred
    out-projections fill remaining PE slots (keeps the HAM clock warm).
  - Normalization: l -> PE partition-broadcast -> reciprocal_approx_fast
    -> DVE mul into yT.
"""
import numpy as np
import ml_dtypes
from contextlib import ExitStack

B, T, C, H, D = 2, 2048, 1024, 16, 64
HLOC = 4            # heads per core
CLOC = HLOC * D     # 256 head-dims per core
VW = HLOC * 65      # v width with ones-columns: 260
N_CORES = 8
TQ = 512            # q tile width
KC = 128            # k chunk
NCC = C // 128      # 8 contraction chunks

_CACHE = {}


def build_nc(with_qk_bias=True):
    import concourse.tile as tile
    from concourse import bacc, mybir

    f32 = mybir.dt.float32
    fmm = mybir.dt.bfloat16
    Exp = mybir.ActivationFunctionType.Exp

    nc = bacc.Bacc("TRN2", target_bir_lowering=False, debug=False,
                   num_devices=N_CORES)
    xT = nc.dram_tensor("xT", [C, T], fmm, kind="ExternalInput").ap()
    wqkT = nc.dram_tensor("wqkT", [C, 2 * CLOC], fmm, kind="ExternalInput").ap()
    wvT = nc.dram_tensor("wvT", [C, VW], fmm, kind="ExternalInput").ap()
    wpT = nc.dram_tensor("wpT", [CLOC, C], fmm, kind="ExternalInput").ap()
    if with_qk_bias:
        bq = nc.dram_tensor("bq", [1, CLOC], fmm, kind="ExternalInput").ap()
        bk = nc.dram_tensor("bk", [1, CLOC], fmm, kind="ExternalInput").ap()
    bv = nc.dram_tensor("bv", [1, VW], fmm, kind="ExternalInput").ap()
    trineg = nc.dram_tensor("trineg", [KC, KC], fmm, kind="ExternalInput").ap()
    ident = nc.dram_tensor("ident", [KC, KC], fmm, kind="ExternalInput").ap()
    po = nc.dram_tensor("po", [T, C], fmm, kind="ExternalOutput").ap()

    with tile.TileContext(nc) as tc, ExitStack() as ctx:
        persist = ctx.enter_context(tc.tile_pool(name="persist", bufs=1))
        pt_pool = ctx.enter_context(tc.tile_pool(name="pt", bufs=5))
        stage = ctx.enter_context(tc.tile_pool(name="stage", bufs=4))
        norm_pool = ctx.enter_context(tc.tile_pool(name="norm", bufs=4))
        ps_small = ctx.enter_context(
            tc.tile_pool(name="ps_small", bufs=2, space="PSUM"))
        ps_st = ctx.enter_context(
            tc.tile_pool(name="ps_st", bufs=2, space="PSUM"))
        ps_yt = ctx.enter_context(
            tc.tile_pool(name="ps_yt", bufs=2, space="PSUM"))

        # ---- persistent SBUF tensors + loads ----
        xT_sb = [persist.tile([128, T], fmm, tag=f"xT{c}", name=f"xT{c}") for c in range(NCC)]
        wqk_sb = [persist.tile([128, 2 * CLOC], fmm, tag=f"wqk{c}", name=f"wqk{c}") for c in range(NCC)]
        wv_sb = [persist.tile([128, VW], fmm, tag=f"wv{c}", name=f"wv{c}") for c in range(NCC)]
        wp_sb = [persist.tile([128, C], fmm, tag=f"wp{m}", name=f"wp{m}") for m in range(2)]
        if with_qk_bias:
            bq_sb = persist.tile([1, CLOC], fmm, tag="bq")
            bk_sb = persist.tile([1, CLOC], fmm, tag="bk")
        # bv / onesrow are [128, *] with only row 0 nonzero: broadcasts run
        # as K=128 matmuls (full-rate stream) instead of K=1 (~0.6 ns/row)
        bv_sb = persist.tile([128, VW], fmm, tag="bv")
        onesrow_sb = persist.tile([128, 128], fmm, tag="onesrow")
        ones_sb = persist.tile([1, TQ], fmm, tag="ones")
        trineg_sb = persist.tile([KC, KC], fmm, tag="trineg")
        ident_sb = persist.tile([KC, KC], fmm, tag="ident")
        qT_sb = [persist.tile([128, T], fmm, tag=f"qT{m}", name=f"qT{m}") for m in range(2)]
        kz_sb = [persist.tile([128, T], fmm, tag=f"kz{h}", name=f"kz{h}") for h in range(HLOC)]
        v_sb = [persist.tile([128, VW], fmm, tag=f"v{t}", name=f"v{t}") for t in range(T // 128)]
        yT_sb = [persist.tile([128, T], fmm, tag=f"yT{m}", name=f"yT{m}") for m in range(2)]

        # zero halves of kz (the other head's partitions stay 0 forever);
        # DVE is idle during the initial DMA wait.  ones via memset, not DMA.
        for h in range(HLOC):
            zsl = slice(64, 128) if h % 2 == 0 else slice(0, 64)
            nc.vector.memset(kz_sb[h][zsl, :], 0.0)
        nc.gpsimd.memset(ones_sb[:], 1.0)
        nc.gpsimd.memset(onesrow_sb[:], 0.0)
        nc.gpsimd.memset(onesrow_sb[0:1, :], 1.0)
        nc.gpsimd.memset(bv_sb[:], 0.0)
        # Input loads split across the three DMA-capable queues (SP / ACT /
        # GPSIMD) so the ~5 MB prologue pull isn't serialized on one ring.
        # sync carries the fused qk weights, scalar the x tile the first
        # projections read, gpsimd the mask constants + v weights; the
        # later x tiles trail on whichever queue frees up first.
        # wqkT host layout: [wq_m0 | wk_m0 | wq_m1 | wk_m1] per chunk, so
        # only the first 256 cols (the m0 half) gate the first projections.
        for c in range(NCC):
            sl = slice(c * 128, (c + 1) * 128)
            nc.sync.dma_start(wqk_sb[c][:, 0:2 * 128], wqkT[sl, 0:2 * 128])
            if c < 4:
                nc.scalar.dma_start(xT_sb[c][:, 0:TQ], xT[sl, 0:TQ])
            else:
                nc.gpsimd.dma_start(xT_sb[c][:, 0:TQ], xT[sl, 0:TQ])
        for c in range(NCC):
            sl = slice(c * 128, (c + 1) * 128)
            eng = nc.scalar if c < 4 else nc.gpsimd
            eng.dma_start(wv_sb[c][:], wvT[sl, :])
        nc.gpsimd.dma_start(bv_sb[0:1, :], bv[:])
        nc.gpsimd.dma_start(trineg_sb[:], trineg[:])
        nc.gpsimd.dma_start(ident_sb[:], ident[:])
        for c in range(NCC):
            sl = slice(c * 128, (c + 1) * 128)
            nc.sync.dma_start(wqk_sb[c][:, 2 * 128:4 * 128],
                              wqkT[sl, 2 * 128:4 * 128])
        # Background tiles stay OFF the scalar queue: ACT issues DMAs
        # in-order ahead of the exps, so anything queued there delays the
        # first softmax.  xT1/xT2 on sync, xT3+wp on gpsimd.
        for c in range(NCC):
            sl = slice(c * 128, (c + 1) * 128)
            nc.sync.dma_start(xT_sb[c][:, TQ:2 * TQ], xT[sl, TQ:2 * TQ])
        for c in range(NCC):
            sl = slice(c * 128, (c + 1) * 128)
            nc.sync.dma_start(xT_sb[c][:, 2 * TQ:3 * TQ], xT[sl, 2 * TQ:3 * TQ])
        for c in range(NCC):
            sl = slice(c * 128, (c + 1) * 128)
            nc.gpsimd.dma_start(xT_sb[c][:, 3 * TQ:4 * TQ], xT[sl, 3 * TQ:4 * TQ])
        for m in range(2):
            nc.gpsimd.dma_start(wp_sb[m][:], wpT[m * 128:(m + 1) * 128, :])
        if with_qk_bias:
            nc.sync.dma_start(bq_sb[:], bq[:])
            nc.sync.dma_start(bk_sb[:], bk[:])

        # ---- interleaved emission: projections / attention / out-proj ----
        # The PE executes its queue in order, so emission order controls PE
        # density.  Attention for q-tile j only needs projections up to
        # t=j, so projections for t=j+1 and the out-projection are woven
        # between attention groups of j to fill PE idle slots.
        def proj_qk(is_k, m, t):
            tsl = slice(t * TQ, (t + 1) * TQ)
            blk = 2 * m + (1 if is_k else 0)   # [q_m0|k_m0|q_m1|k_m1]
            wsl = slice(blk * 128, (blk + 1) * 128)
            msl = slice(m * 128, (m + 1) * 128)
            ps = ps_small.tile([128, TQ], f32, tag="ps_small")
            for c in range(NCC):
                nc.tensor.matmul(ps[:], wqk_sb[c][:, wsl], xT_sb[c][:, tsl],
                                 start=(c == 0),
                                 stop=(c == NCC - 1 and not with_qk_bias))
            if with_qk_bias:
                b_sb = bk_sb if is_k else bq_sb
                nc.tensor.matmul(ps[:], b_sb[0:1, msl], ones_sb[0:1, :],
                                 start=False, stop=True)
            if is_k:
                nc.vector.tensor_copy(kz_sb[2 * m][0:64, tsl], ps[0:64, :])
                nc.vector.tensor_copy(kz_sb[2 * m + 1][64:128, tsl],
                                      ps[64:128, :])
            else:
                nc.vector.tensor_copy(qT_sb[m][:, tsl], ps[:])

        def proj_v(tt):
            ttsl = slice(tt * 128, tt * 128 + 128)
            ps = ps_small.tile([128, VW], f32, tag="ps_small")
            for c in range(NCC):
                nc.tensor.matmul(ps[:], xT_sb[c][:, ttsl], wv_sb[c][:],
                                 start=(c == 0), stop=False)
            # always emitted: supplies the ones-columns (+ v bias)
            nc.tensor.matmul(ps[:], onesrow_sb[:], bv_sb[:],
                             start=False, stop=True)
            nc.vector.tensor_copy(v_sb[tt][:], ps[:])

        def proj_pieces(t):
            out = []
            for is_k in (False, True):
                for m in range(2):
                    out.append(lambda k=is_k, mm=m: proj_qk(k, mm, t))
            for tt in range(t * 4, t * 4 + 4):
                out.append(lambda x=tt: proj_v(x))
            return out

        def outproj_piece(tt, do, alt=False, act_copy=False):
            ttsl = slice(tt * 128, (tt + 1) * 128)
            dsl = slice(do * TQ, (do + 1) * TQ)
            if alt:   # epilogue: borrow the idle attention PSUM/ACT
                ops = ps_st.tile([128, TQ], f32, tag="st")
            else:
                ops = ps_small.tile([128, TQ], f32, tag="ps_small")
            for m2 in range(2):
                nc.tensor.matmul(ops[:], yT_sb[m2][:, ttsl],
                                 wp_sb[m2][:, dsl],
                                 start=(m2 == 0), stop=(m2 == 1))
            so = stage.tile([128, TQ], fmm, tag="so2" if alt else "so")
            if alt:
                nc.scalar.copy(so[:], ops[:])
                nc.gpsimd.dma_start(po[ttsl, dsl], so[:])
            elif act_copy:
                nc.scalar.copy(so[:], ops[:])
                nc.sync.dma_start(po[ttsl, dsl], so[:])
            else:
                nc.vector.tensor_copy(so[:], ops[:])
                nc.sync.dma_start(po[ttsl, dsl], so[:])

        def outproj_pieces(j, act_every=0):
            ts = [(tt, do) for tt in range(4 * j, 4 * j + 4) for do in range(2)]
            return [lambda t=tt, d=do, a=(act_every and pi % act_every == act_every - 1):
                    outproj_piece(t, d, act_copy=a)
                    for pi, (tt, do) in enumerate(ts)]

        def s_group(j, h, kcs):
            """Emit S matmuls for a k-chunk pair; return (st_tile, info).

            K=128 against zero-padded kz.  Diagonal chunks are split so the
            128-wide diagonal block forms its own accumulation group with a
            trineg (-240 upper triangle) matmul-add before stop.
            """
            m = h // 2
            st = ps_st.tile([128, 1024], f32, tag="st")
            info = []
            for i, kc in enumerate(kcs):
                coff = max(0, kc * KC - j * TQ)   # causal column offset
                lo = i * TQ + coff
                qlo = j * TQ + coff
                if kc >= 4 * j:   # diagonal chunk
                    nc.tensor.matmul(
                        st[:, lo:lo + KC],
                        kz_sb[h][:, kc * KC:(kc + 1) * KC],
                        qT_sb[m][:, qlo:qlo + KC],
                        start=True, stop=False)
                    nc.tensor.matmul(
                        st[:, lo:lo + KC], ident_sb[:], trineg_sb[:],
                        start=False, stop=True)
                    if (i + 1) * TQ > lo + KC:
                        nc.tensor.matmul(
                            st[:, lo + KC:(i + 1) * TQ],
                            kz_sb[h][:, kc * KC:(kc + 1) * KC],
                            qT_sb[m][:, qlo + KC:(j + 1) * TQ],
                            start=True, stop=True)
                else:
                    nc.tensor.matmul(
                        st[:, lo:(i + 1) * TQ],
                        kz_sb[h][:, kc * KC:(kc + 1) * KC],
                        qT_sb[m][:, qlo:(j + 1) * TQ],
                        start=True, stop=True)
                info.append((i, kc, coff))
            return st, info

        def pv_group(j, h, st, info, yt, nk, split_exp=False):
            """exp + PV matmuls for a prepared S group (mask already in st).

            split_exp: emit one exp per chunk instead of merged runs so the
            first PV matmul's dependency resolves ~0.5us earlier (used for
            the first group after a head transition, where the software
            pipeline has the least slack).
            """
            pt = pt_pool.tile([128, 1024], fmm, tag="pt")
            runs = []
            for i, kc, coff in info:
                lo, hi = i * TQ + coff, (i + 1) * TQ
                if not split_exp and runs and runs[-1][1] == lo:
                    runs[-1][1] = hi
                else:
                    runs.append([lo, hi])
            for lo, hi in runs:
                nc.scalar.activation(pt[:, lo:hi], st[:, lo:hi], Exp, scale=0.125)
            for i, kc, coff in info:
                lo = i * TQ + coff
                nc.tensor.matmul(
                    yt[0:65, coff:TQ] if coff else yt[:],
                    v_sb[kc][:, h * 65:(h + 1) * 65],
                    pt[:, lo:(i + 1) * TQ],
                    start=(kc == 0), stop=(kc == nk - 1))

        # l tiles are [128, TQ] with rows 1..127 pre-zeroed (once, below) so
        # the broadcast matmul runs K=128 full-rate; only row 0 is rewritten
        for _ in range(4):
            lz = norm_pool.tile([128, TQ], fmm, tag="l", name="lz")
            nc.vector.memset(lz[:], 0.0)

        def norm_a(yt, eng=None):
            """Pull the softmax denominator out of PSUM right away."""
            l_sb = norm_pool.tile([128, TQ], fmm, tag="l")
            if eng == "scalar":
                nc.scalar.copy(l_sb[0:1, :], yt[64:65, :])
            else:
                nc.vector.tensor_copy(l_sb[0:1, :], yt[64:65, :])
            return l_sb

        def norm_b(j, h, yt, l_sb):
            """yT[h slice, j] = yt[0:64] * broadcast(1/l)."""
            m, pr = h // 2, (h % 2) * 64
            bc_ps = ps_small.tile([64, TQ], f32, tag="ps_small")
            nc.tensor.matmul(bc_ps[:], onesrow_sb[:, 0:64], l_sb[:],
                             start=True, stop=True)
            bc_sb = stage.tile([64, TQ], f32, tag="bc")
            nc.vector.reciprocal_approx_fast(bc_sb[:], bc_ps[:])
            nc.vector.tensor_mul(yT_sb[m][pr:pr + 64, j * TQ:(j + 1) * TQ],
                                 yt[0:64, :], bc_sb[:])

        # prologue: only what the first attention groups need (m0 heads,
        # first two v tiles); the rest weaves into j=0 as extras so S(h0)
        # starts as soon as the critical DMA lands
        proj_qk(False, 0, 0)
        proj_qk(True, 0, 0)
        proj_v(0)
        proj_v(1)

        for j in range(T // TQ):
            nk = 4 * (j + 1)
            groups = []
            for h in range(HLOC):
                for k0 in range(0, nk, 2):
                    groups.append((h, [k for k in (k0, k0 + 1) if k < nk],
                                   k0 == 0))
            extras = []
            if j == 0:
                extras += [lambda: proj_v(2), lambda: proj_v(3),
                           lambda: proj_qk(False, 1, 0),
                           lambda: proj_qk(True, 1, 0)]
            if j + 1 < T // TQ:
                extras += proj_pieces(j + 1)
            if j == T // TQ - 1:
                # all deferred out-projections: PE filler for the
                # ACT-paced final block (keeps HAM warm)
                for jj in range(T // TQ - 1):
                    extras += outproj_pieces(jj)
            ei = 0           # extras emitted so far
            yts = {}
            pending = None   # (h, st, info, first) awaiting exp/PV
            norm_w = []      # [age, h, yt, l_sb] awaiting broadcast/mul
            for gi, (h, kcs, first) in enumerate(groups):
                for e in norm_w:
                    e[0] += 1
                if norm_w and norm_w[0][0] >= 3:
                    _, nh, nyt, nl = norm_w.pop(0)
                    norm_b(j, nh, nyt, nl)
                if h not in yts:
                    yts[h] = ps_yt.tile([65, TQ], f32, tag="yt",
                                        name=f"yt{j}_{h}")
                st, info = s_group(j, h, kcs)
                if pending is not None:
                    ph, pst, pinfo, pfirst = pending
                    pv_group(j, ph, pst, pinfo, yts[ph], nk, pfirst)
                    if ph != h:
                        yt = yts.pop(ph)
                        norm_w.append([0, ph, yt, norm_a(yt)])
                pending = (h, st, info, first)
                want = (gi + 1) * len(extras) // len(groups)
                while ei < want:
                    extras[ei]()
                    ei += 1
            ph, pst, pinfo, pfirst = pending
            pv_group(j, ph, pst, pinfo, yts[ph], nk, pfirst)
            yt = yts.pop(ph)
            last_j = j == T // TQ - 1
            norm_w.append([0, ph, yt, norm_a(yt, "scalar" if last_j else None)])
            while ei < len(extras):
                extras[ei]()
                ei += 1
            for _, nh, nyt, nl in norm_w:
                norm_b(j, nh, nyt, nl)

        for pi, (tt, do) in enumerate(
                [(t, d) for t in range(4 * (T // TQ - 1), 4 * (T // TQ))
                 for d in range(2)]):   # epilogue
            outproj_piece(tt, do, alt=(pi % 2 == 1))
    nc.compile()
    return nc


def make_in_maps(x, Wq, bq, Wk, bk, Wv, bv, Wp, bp):
    bf16 = ml_dtypes.bfloat16
    x = np.asarray(x, np.float32)
    Wq, Wk, Wv, Wp = (np.asarray(w, np.float32) for w in (Wq, Wk, Wv, Wp))
    bq, bk, bv = (np.asarray(b, np.float32) for b in (bq, bk, bv))

    kp = np.arange(KC)[:, None]
    qf = np.arange(KC)[None, :]
    trineg = np.where(kp > qf, -240.0, 0.0).astype(bf16)
    ident = np.eye(KC, dtype=bf16)
    with_qk_bias = bool(np.any(bq) or np.any(bk))

    in_maps = []
    for core in range(N_CORES):
        b = core // 4
        hg = core % 4
        rows = slice(hg * CLOC, (hg + 1) * CLOC)
        wv_aug = np.zeros((C, VW), np.float32)
        bv_aug = np.zeros((1, VW), np.float32)
        for h in range(HLOC):
            wsl = slice(hg * CLOC + h * D, hg * CLOC + (h + 1) * D)
            wv_aug[:, h * 65:h * 65 + D] = Wv[wsl, :].T
            bv_aug[0, h * 65:h * 65 + D] = bv[wsl]
            bv_aug[0, h * 65 + D] = 1.0
        im = {
            "xT": np.ascontiguousarray(x[b].T).astype(bf16),
            "wqkT": np.ascontiguousarray(
                np.concatenate([Wq[rows, :].T[:, 0:128], Wk[rows, :].T[:, 0:128],
                                Wq[rows, :].T[:, 128:256], Wk[rows, :].T[:, 128:256]],
                               axis=1)
            ).astype(bf16),
            "wvT": wv_aug.astype(bf16),
            "wpT": np.ascontiguousarray(Wp[:, rows].T).astype(bf16),
            "bv": bv_aug.astype(bf16),
            "trineg": trineg,
            "ident": ident,
        }
        if with_qk_bias:
            im["bq"] = np.ascontiguousarray(bq[rows][None, :]).astype(bf16)
            im["bk"] = np.ascontiguousarray(bk[rows][None, :]).astype(bf16)
        in_maps.append(im)
    return in_maps


def kernel(x, Wq, bq, Wk, bk, Wv, bv, Wp, bp):
    from concourse.bass_utils import run_bass_kernel_spmd

    with_qk_bias = bool(np.any(np.asarray(bq)) or np.any(np.asarray(bk)))
    key = ("nc", with_qk_bias)
    if key not in _CACHE:
        _CACHE[key] = build_nc(with_qk_bias)
    nc = _CACHE[key]
    in_maps = make_in_maps(x, Wq, bq, Wk, bk, Wv, bv, Wp, bp)
    res = run_bass_kernel_spmd(nc, in_maps, core_ids=list(range(N_CORES)))
    out = np.zeros((B, T, C), np.float32)
    for core in range(N_CORES):
        out[core // 4] += np.asarray(res.results[core]["po"], np.float32)
    out += np.asarray(bp, np.float32)[None, None, :]
    return out
